# revision 8
# baseline (speedup 1.0000x reference)
"""Trainium2 Bass kernel for nn_ConvexReLU.

Math: out[i,m] = sum_{j,k,l} G[j,k] * x[i,k,l] * (v-w)[j,l,m]

Reassociated as:
    d = v - w                              (host, elementwise)
    T[k,l,m]   = sum_j G[j,k] * d[j,l,m]   (device matmul, 68.7 GFLOP)
    out[i,m]   = sum_{k,l} x[i,k,l] * T[k,l,m]   (device matmul, 17.2 GFLOP)

Sharding: split l (in_dim, 256) across 8 cores (32 each). Each core computes
a full-shape (out_dim, batch) partial; host sums the 8 partials.

Device layout per core:
    g  : (1024 j, 1024 k)      full G, replicated
    d  : (1024 j, 32 l, 128 m) l-shard of v-w
    xt : (32 l, 1024 k, 256 i) l-shard of x, transposed on host
    out: (128 m, 256 i)        partial of out^T

Matmuls run as float32r (fp22 multiply, fp32 accumulate): full PE rate for
moving dims >= 256, ~2^-14 relative precision.
"""

import os
import sys

import numpy as np

for _p in ("/opt/trn_rl_repo", "/root/.axon_site/_ro/trn_rl_repo"):
    if os.path.isdir(_p) and _p not in sys.path:
        sys.path.insert(0, _p)

import concourse.bass as bass
import concourse.bacc as bacc
import concourse.mybir as mybir
from concourse.bass_utils import run_bass_kernel_spmd
from concourse.tile import TileContext

B, J, K, L, M = 256, 1024, 1024, 256, 128
NCORES = 8
LC = L // NCORES          # 32 l-values per core
NPG = 8                   # l-groups per core
LG = LC // NPG            # 4 l-values per group
NKT = K // 128            # 8 k-tiles
NJC = J // 128            # 8 j-chunks

F32 = mybir.dt.float32
F32R = mybir.dt.float32r
BF16 = mybir.dt.bfloat16

DTYPE = os.environ.get("BASS_KERNEL_DTYPE", "f32r")


def build_nc(dtype_name: str = DTYPE) -> bass.Bass:
    io_dt = BF16 if dtype_name == "bf16" else F32R

    nc = bacc.Bacc(None, debug=False)

    g = nc.declare_dram_parameter("g", [J, K], io_dt, isOutput=False)
    d = nc.declare_dram_parameter("d", [J, LC, M], io_dt, isOutput=False)
    xt = nc.declare_dram_parameter("xt", [LC, K, B], io_dt, isOutput=False)
    out = nc.declare_dram_parameter("out", [M, B], F32, isOutput=True)

    g_r = g.rearrange("(jc p) k -> p jc k", p=128)
    d_r = d.rearrange("(jc p) l m -> p jc (l m)", p=128)
    xt_r = xt.rearrange("l (kt p) i -> l p kt i", p=128)

    with TileContext(nc) as tc:
        with (
            tc.tile_pool(name="gpool", bufs=1) as gpool,
            tc.tile_pool(name="dpool", bufs=2) as dpool,
            tc.tile_pool(name="tpool", bufs=2) as tpool,
            tc.tile_pool(name="xpool", bufs=6) as xpool,
            tc.tile_pool(name="opool", bufs=1) as opool,
            tc.tile_pool(name="ps1", bufs=3, space="PSUM") as ps1,
            tc.tile_pool(name="pso", bufs=1, space="PSUM") as pso,
        ):
            g_sb = gpool.tile([128, NJC, K], io_dt)
            nc.sync.dma_start(out=g_sb[:], in_=g_r)

            out_ps = pso.tile([M, B], F32)

            mm2 = 0
            total_mm2 = NPG * LG * NKT

            for pg in range(NPG):
                # ---- stage 1: T[k, (l,m)] for this l-group ----
                d_sb = dpool.tile([128, NJC, LG * M], io_dt, tag="d")
                nc.sync.dma_start(
                    out=d_sb[:],
                    in_=d_r[:, :, pg * LG * M : (pg + 1) * LG * M],
                )

                t_sb = tpool.tile([128, NKT, LG * M], io_dt, tag="t")
                for kt in range(NKT):
                    p1 = ps1.tile([128, LG * M], F32, tag="p1")
                    for jc in range(NJC):
                        nc.tensor.matmul(
                            p1[:],
                            g_sb[:, jc, kt * 128 : (kt + 1) * 128],
                            d_sb[:, jc, :],
                            start=(jc == 0),
                            stop=(jc == NJC - 1),
                        )
                    nc.vector.tensor_copy(out=t_sb[:, kt, :], in_=p1[:])

                # ---- stage 2: out^T += T^T-slices @ x^T-slices ----
                for dl in range(LG):
                    x_sb = xpool.tile([128, NKT, B], io_dt, tag="x")
                    nc.sync.dma_start(out=x_sb[:], in_=xt_r[pg * LG + dl])
                    for kt in range(NKT):
                        nc.tensor.matmul(
                            out_ps[:],
                            t_sb[:, kt, dl * M : (dl + 1) * M],
                            x_sb[:, kt, :],
                            start=(mm2 == 0),
                            stop=(mm2 == total_mm2 - 1),
                            skip_group_check=True,
                        )
                        mm2 += 1

            out_sb = opool.tile([M, B], F32)
            nc.vector.tensor_copy(out=out_sb[:], in_=out_ps[:])
            nc.sync.dma_start(out=out[:], in_=out_sb[:])

    nc.finalize()
    return nc


_NC_CACHE: dict[str, bass.Bass] = {}


def _get_nc(dtype_name: str = DTYPE) -> bass.Bass:
    if dtype_name not in _NC_CACHE:
        _NC_CACHE[dtype_name] = build_nc(dtype_name)
    return _NC_CACHE[dtype_name]


def make_in_maps(x, G, v, w, dtype_name: str = DTYPE):
    x = np.asarray(x, dtype=np.float32)
    G = np.asarray(G, dtype=np.float32)
    v = np.asarray(v, dtype=np.float32)
    w = np.asarray(w, dtype=np.float32)

    d_full = v - w  # (J, L, M)

    if dtype_name == "bf16":
        import ml_dtypes

        np_dt = ml_dtypes.bfloat16
    else:
        np_dt = np.float32

    G_io = np.ascontiguousarray(G.astype(np_dt))
    in_maps = []
    for c in range(NCORES):
        ls = slice(c * LC, (c + 1) * LC)
        d_c = np.ascontiguousarray(d_full[:, ls, :].astype(np_dt))
        # x (B,K,L) -> xt (LC, K, B)
        xt_c = np.ascontiguousarray(x[:, :, ls].transpose(2, 1, 0).astype(np_dt))
        in_maps.append({"g": G_io, "d": d_c, "xt": xt_c})
    return in_maps


def kernel(x, G, v, w):
    nc = _get_nc()
    in_maps = make_in_maps(x, G, v, w)
    res = run_bass_kernel_spmd(nc, in_maps, core_ids=list(range(NCORES)))
    acc = np.zeros((M, B), dtype=np.float64)
    for r in res.results:
        acc += r["out"].astype(np.float64)
    return np.ascontiguousarray(acc.T.astype(np.float32))


# revision 13
# speedup vs baseline: 1.0645x; 1.0645x over previous
"""Trainium2 Bass kernel for nn_ConvexReLU.

Math: out[i,m] = sum_{j,k,l} G[j,k] * x[i,k,l] * (v-w)[j,l,m]

Reassociated as:
    d = v - w                              (host, elementwise)
    T[k,l,m]   = sum_j G[j,k] * d[j,l,m]   (device matmul, 68.7 GFLOP)
    out[i,m]   = sum_{k,l} x[i,k,l] * T[k,l,m]   (device matmul, 17.2 GFLOP)

Sharding: split l (in_dim, 256) across 8 cores (32 each). Each core computes
a full-shape (out_dim, batch) partial; host sums the 8 partials.

Device layout per core:
    g  : (1024 j, 1024 k)      full G, replicated
    d  : (1024 j, 32 l, 128 m) l-shard of v-w
    xt : (32 l, 1024 k, 256 i) l-shard of x, transposed on host
    out: (128 m, 256 i)        partial of out^T

Matmuls run as float32r (fp22 multiply, fp32 accumulate): full PE rate for
moving dims >= 256, ~2^-14 relative precision.
"""

import os
import sys

import numpy as np

for _p in ("/opt/trn_rl_repo", "/root/.axon_site/_ro/trn_rl_repo"):
    if os.path.isdir(_p) and _p not in sys.path:
        sys.path.insert(0, _p)

import concourse.bass as bass
import concourse.bacc as bacc
import concourse.mybir as mybir
from concourse.bass_utils import run_bass_kernel_spmd
from concourse.tile import TileContext

B, J, K, L, M = 256, 1024, 1024, 256, 128
NCORES = 8
LC = L // NCORES          # 32 l-values per core
NPG = 8                   # l-groups per core
LG = LC // NPG            # 4 l-values per group
NKT = K // 128            # 8 k-tiles
NJC = J // 128            # 8 j-chunks

F32 = mybir.dt.float32
F32R = mybir.dt.float32r
BF16 = mybir.dt.bfloat16

DTYPE = os.environ.get("BASS_KERNEL_DTYPE", "f32r")


def _dtypes(dtype_name: str):
    # (g/d stage-1 dtype, t/x stage-2 dtype). Stage-2 must be dtype-uniform:
    # f32r stationary + bf16 moving takes the explicit-LDWEIGHTS path, which
    # yields all-zero HW output for f32r weights.
    if dtype_name == "bf16":
        return BF16, BF16
    if dtype_name == "mixed":
        return F32R, BF16
    return F32R, F32R


def build_nc(dtype_name: str = DTYPE) -> bass.Bass:
    gd_dt, s2_dt = _dtypes(dtype_name)

    nc = bacc.Bacc(None, debug=False)

    g = nc.declare_dram_parameter("g", [J, K], gd_dt, isOutput=False)
    d = nc.declare_dram_parameter("d", [J, LC, M], gd_dt, isOutput=False)
    xt = nc.declare_dram_parameter("xt", [LC, K, B], s2_dt, isOutput=False)
    out = nc.declare_dram_parameter("out", [M, B], F32, isOutput=True)

    g_r = g.rearrange("(jc p) k -> p jc k", p=128)
    d_r = d.rearrange("(jc p) l m -> p jc (l m)", p=128)
    xt_r = xt.rearrange("l (kt p) i -> l p kt i", p=128)

    with TileContext(nc) as tc:
        with (
            tc.tile_pool(name="gpool", bufs=1) as gpool,
            tc.tile_pool(name="dpool", bufs=2) as dpool,
            tc.tile_pool(name="tpool", bufs=2) as tpool,
            tc.tile_pool(name="xpool", bufs=6) as xpool,
            tc.tile_pool(name="opool", bufs=1) as opool,
            tc.tile_pool(name="ps1", bufs=4, space="PSUM") as ps1,
            tc.tile_pool(name="pso", bufs=1, space="PSUM") as pso,
        ):
            # per-jc DMAs so the first matmuls unblock after ~0.5 MB, not 4 MB
            g_sb = gpool.tile([128, NJC, K], gd_dt)
            for jc in range(NJC):
                nc.sync.dma_start(out=g_sb[:, jc, :], in_=g_r[:, jc, :])

            out_ps = pso.tile([M, B], F32)

            mm2 = 0
            total_mm2 = NPG * LG * NKT

            for pg in range(NPG):
                # ---- stage 1: T[k, (l,m)] for this l-group ----
                d_sb = dpool.tile([128, NJC, LG * M], gd_dt, tag="d")
                for jc in range(NJC):
                    nc.sync.dma_start(
                        out=d_sb[:, jc, :],
                        in_=d_r[:, jc, pg * LG * M : (pg + 1) * LG * M],
                    )

                t_sb = tpool.tile([128, NKT, LG * M], s2_dt, tag="t")
                for kt in range(NKT):
                    p1 = ps1.tile([128, LG * M], F32, tag="p1")
                    for jc in range(NJC):
                        nc.tensor.matmul(
                            p1[:],
                            g_sb[:, jc, kt * 128 : (kt + 1) * 128],
                            d_sb[:, jc, :],
                            start=(jc == 0),
                            stop=(jc == NJC - 1),
                        )
                    nc.vector.tensor_copy(out=t_sb[:, kt, :], in_=p1[:])

                # ---- stage 2: out^T += T^T-slices @ x^T-slices ----
                for dl in range(LG):
                    x_sb = xpool.tile([128, NKT, B], s2_dt, tag="x")
                    nc.sync.dma_start(out=x_sb[:], in_=xt_r[pg * LG + dl])
                    for kt in range(NKT):
                        nc.tensor.matmul(
                            out_ps[:],
                            t_sb[:, kt, dl * M : (dl + 1) * M],
                            x_sb[:, kt, :],
                            start=(mm2 == 0),
                            stop=(mm2 == total_mm2 - 1),
                            skip_group_check=True,
                        )
                        mm2 += 1

            out_sb = opool.tile([M, B], F32)
            nc.vector.tensor_copy(out=out_sb[:], in_=out_ps[:])
            nc.sync.dma_start(out=out[:], in_=out_sb[:])

    nc.finalize()
    return nc


_NC_CACHE: dict[str, bass.Bass] = {}


def _get_nc(dtype_name: str = DTYPE) -> bass.Bass:
    if dtype_name not in _NC_CACHE:
        _NC_CACHE[dtype_name] = build_nc(dtype_name)
    return _NC_CACHE[dtype_name]


def make_in_maps(x, G, v, w, dtype_name: str = DTYPE):
    x = np.asarray(x, dtype=np.float32)
    G = np.asarray(G, dtype=np.float32)
    v = np.asarray(v, dtype=np.float32)
    w = np.asarray(w, dtype=np.float32)

    d_full = v - w  # (J, L, M)

    import ml_dtypes

    if dtype_name == "bf16":
        gd_np, x_np = ml_dtypes.bfloat16, ml_dtypes.bfloat16
    elif dtype_name == "mixed":
        gd_np, x_np = np.float32, ml_dtypes.bfloat16
    else:
        gd_np, x_np = np.float32, np.float32

    G_io = np.ascontiguousarray(G.astype(gd_np))
    in_maps = []
    for c in range(NCORES):
        ls = slice(c * LC, (c + 1) * LC)
        d_c = np.ascontiguousarray(d_full[:, ls, :].astype(gd_np))
        # x (B,K,L) -> xt (LC, K, B)
        xt_c = np.ascontiguousarray(x[:, :, ls].transpose(2, 1, 0).astype(x_np))
        in_maps.append({"g": G_io, "d": d_c, "xt": xt_c})
    return in_maps


def kernel(x, G, v, w):
    nc = _get_nc()
    in_maps = make_in_maps(x, G, v, w)
    res = run_bass_kernel_spmd(nc, in_maps, core_ids=list(range(NCORES)))
    acc = np.zeros((M, B), dtype=np.float64)
    for r in res.results:
        acc += r["out"].astype(np.float64)
    return np.ascontiguousarray(acc.T.astype(np.float32))


# revision 15
# speedup vs baseline: 1.1424x; 1.0732x over previous
"""Trainium2 Bass kernel for nn_ConvexReLU.

Math: out[i,m] = sum_{j,k,l} G[j,k] * x[i,k,l] * (v-w)[j,l,m]

Reassociated as:
    d = v - w                              (host, elementwise)
    T[k,l,m]   = sum_j G[j,k] * d[j,l,m]   (device matmul, 68.7 GFLOP)
    out[i,m]   = sum_{k,l} x[i,k,l] * T[k,l,m]   (device matmul, 17.2 GFLOP)

Sharding: split l (in_dim, 256) across 8 cores (32 each). Each core computes
a full-shape (out_dim, batch) partial; host sums the 8 partials.

Device layout per core:
    g  : (1024 j, 1024 k)      full G, replicated
    d  : (1024 j, 32 l, 128 m) l-shard of v-w
    xt : (32 l, 1024 k, 256 i) l-shard of x, transposed on host
    out: (128 m, 256 i)        partial of out^T

Matmuls run as float32r (fp22 multiply, fp32 accumulate): full PE rate for
moving dims >= 256, ~2^-14 relative precision.
"""

import os
import sys

import numpy as np

for _p in ("/opt/trn_rl_repo", "/root/.axon_site/_ro/trn_rl_repo"):
    if os.path.isdir(_p) and _p not in sys.path:
        sys.path.insert(0, _p)

import concourse.bass as bass
import concourse.bacc as bacc
import concourse.mybir as mybir
from concourse.bass_utils import run_bass_kernel_spmd
from concourse.tile import TileContext

B, J, K, L, M = 256, 1024, 1024, 256, 128
NCORES = 8
LC = L // NCORES          # 32 l-values per core
NPG = 8                   # l-groups per core
LG = LC // NPG            # 4 l-values per group
NKT = K // 128            # 8 k-tiles
NJC = J // 128            # 8 j-chunks

F32 = mybir.dt.float32
F32R = mybir.dt.float32r
BF16 = mybir.dt.bfloat16

DTYPE = os.environ.get("BASS_KERNEL_DTYPE", "f32r")


def _dtypes(dtype_name: str):
    # (g/d stage-1 dtype, t/x stage-2 dtype). Stage-2 must be dtype-uniform:
    # f32r stationary + bf16 moving takes the explicit-LDWEIGHTS path, which
    # yields all-zero HW output for f32r weights.
    if dtype_name == "bf16":
        return BF16, BF16
    if dtype_name == "mixed":
        return F32R, BF16
    return F32R, F32R


def build_nc(dtype_name: str = DTYPE) -> bass.Bass:
    gd_dt, s2_dt = _dtypes(dtype_name)

    nc = bacc.Bacc(None, debug=False)

    g = nc.declare_dram_parameter("g", [J, K], gd_dt, isOutput=False)
    d = nc.declare_dram_parameter("d", [J, LC, M], gd_dt, isOutput=False)
    xt = nc.declare_dram_parameter("xt", [LC, K, B], s2_dt, isOutput=False)
    out = nc.declare_dram_parameter("out", [M, B], F32, isOutput=True)

    g_r = g.rearrange("(jc p) k -> p jc k", p=128)
    d_r = d.rearrange("(jc p) l m -> p jc (l m)", p=128)
    xt_r = xt.rearrange("l (kt p) i -> l p kt i", p=128)

    with TileContext(nc) as tc:
        with (
            tc.tile_pool(name="gpool", bufs=1) as gpool,
            tc.tile_pool(name="dpool", bufs=2) as dpool,
            tc.tile_pool(name="tpool", bufs=2) as tpool,
            tc.tile_pool(name="xpool", bufs=6) as xpool,
            tc.tile_pool(name="opool", bufs=1) as opool,
            tc.tile_pool(name="ps1", bufs=4, space="PSUM") as ps1,
            tc.tile_pool(name="pso", bufs=1, space="PSUM") as pso,
        ):
            # per-jc DMAs so the first matmuls unblock after ~0.75 MB, not 6 MB.
            # pg=0's d chunks are interleaved with g chunks: stage-1 consumes
            # (g[jc], d[jc]) pairs in jc order.
            g_sb = gpool.tile([128, NJC, K], gd_dt)
            d_sb0 = dpool.tile([128, NJC, LG * M], gd_dt, tag="d")
            for jc in range(NJC):
                nc.sync.dma_start(out=g_sb[:, jc, :], in_=g_r[:, jc, :])
                nc.sync.dma_start(
                    out=d_sb0[:, jc, :], in_=d_r[:, jc, 0 : LG * M]
                )

            out_ps = pso.tile([M, B], F32)

            mm2 = 0
            total_mm2 = NPG * LG * NKT
            KH = 4  # kt-tiles per half; psum: KH live stage-1 banks + out

            for pg in range(NPG):
                # ---- stage 1: T[k, (l,m)] for this l-group ----
                if pg == 0:
                    d_sb = d_sb0
                else:
                    d_sb = dpool.tile([128, NJC, LG * M], gd_dt, tag="d")
                    for jc in range(NJC):
                        nc.sync.dma_start(
                            out=d_sb[:, jc, :],
                            in_=d_r[:, jc, pg * LG * M : (pg + 1) * LG * M],
                        )

                t_sb = tpool.tile([128, NKT, LG * M], s2_dt, tag="t")
                for half in range(NKT // KH):
                    p1s = [ps1.tile([128, LG * M], F32, tag="p1", name=f"p1_{pg}_{half}_{i}") for i in range(KH)]
                    # jc-outer: each (g[jc], d[jc]) pair is fully consumed as
                    # soon as its DMA lands -> PE starts ~2.5us into the kernel
                    for jc in range(NJC):
                        for kt2 in range(KH):
                            kt = half * KH + kt2
                            nc.tensor.matmul(
                                p1s[kt2][:],
                                g_sb[:, jc, kt * 128 : (kt + 1) * 128],
                                d_sb[:, jc, :],
                                start=(jc == 0),
                                stop=(jc == NJC - 1),
                                skip_group_check=True,
                            )
                    for kt2 in range(KH):
                        kt = half * KH + kt2
                        nc.vector.tensor_copy(out=t_sb[:, kt, :], in_=p1s[kt2][:])

                # ---- stage 2: out^T += T^T-slices @ x^T-slices ----
                for dl in range(LG):
                    x_sb = xpool.tile([128, NKT, B], s2_dt, tag="x")
                    nc.sync.dma_start(out=x_sb[:], in_=xt_r[pg * LG + dl])
                    for kt in range(NKT):
                        nc.tensor.matmul(
                            out_ps[:],
                            t_sb[:, kt, dl * M : (dl + 1) * M],
                            x_sb[:, kt, :],
                            start=(mm2 == 0),
                            stop=(mm2 == total_mm2 - 1),
                            skip_group_check=True,
                        )
                        mm2 += 1

            out_sb = opool.tile([M, B], F32)
            nc.vector.tensor_copy(out=out_sb[:], in_=out_ps[:])
            nc.sync.dma_start(out=out[:], in_=out_sb[:])

    nc.finalize()
    return nc


_NC_CACHE: dict[str, bass.Bass] = {}


def _get_nc(dtype_name: str = DTYPE) -> bass.Bass:
    if dtype_name not in _NC_CACHE:
        _NC_CACHE[dtype_name] = build_nc(dtype_name)
    return _NC_CACHE[dtype_name]


def make_in_maps(x, G, v, w, dtype_name: str = DTYPE):
    x = np.asarray(x, dtype=np.float32)
    G = np.asarray(G, dtype=np.float32)
    v = np.asarray(v, dtype=np.float32)
    w = np.asarray(w, dtype=np.float32)

    d_full = v - w  # (J, L, M)

    import ml_dtypes

    if dtype_name == "bf16":
        gd_np, x_np = ml_dtypes.bfloat16, ml_dtypes.bfloat16
    elif dtype_name == "mixed":
        gd_np, x_np = np.float32, ml_dtypes.bfloat16
    else:
        gd_np, x_np = np.float32, np.float32

    G_io = np.ascontiguousarray(G.astype(gd_np))
    in_maps = []
    for c in range(NCORES):
        ls = slice(c * LC, (c + 1) * LC)
        d_c = np.ascontiguousarray(d_full[:, ls, :].astype(gd_np))
        # x (B,K,L) -> xt (LC, K, B)
        xt_c = np.ascontiguousarray(x[:, :, ls].transpose(2, 1, 0).astype(x_np))
        in_maps.append({"g": G_io, "d": d_c, "xt": xt_c})
    return in_maps


def kernel(x, G, v, w):
    nc = _get_nc()
    in_maps = make_in_maps(x, G, v, w)
    res = run_bass_kernel_spmd(nc, in_maps, core_ids=list(range(NCORES)))
    acc = np.zeros((M, B), dtype=np.float64)
    for r in res.results:
        acc += r["out"].astype(np.float64)
    return np.ascontiguousarray(acc.T.astype(np.float32))
